# revision 6
# baseline (speedup 1.0000x reference)
"""Locally-connected conv (per-location weights) + ReLU on 8 Trainium2 cores.

Problem: x (B=64, Cin=64, H=64, W=64), weights (H, W, Cout=64, Cin=64, 3, 3)
  out[r,a,i,j] = relu( sum_{b,c,d} weights[i,j,a,b,c,d] * xpad[r,b,i+c,j+d] )

Sharding: data-parallel over H — core cid owns output rows i in [8*cid, 8*cid+8).
No collectives; pure SPMD with per-core input slices.

Device strategy (per core):
  - Host pre-packs weights into contraction-major tiles so every DMA has
    multi-KB contiguous partition lines (full HBM bandwidth).
  - x is padded/transposed on host to x_t[b, u, r, v] (u=h+1, v=w+1 padded
    planes); pairs of planes are stacked into 128-partition SBUF tiles so a
    single K=128 matmul contracts Cin x 2 vertical taps at once.
  - Per output row-pair and 16-column block: 3 dual-tap (K=128) + pairs of
    single-tap (K=64, opposite partition halves, run concurrently on the PE
    via row-group tiling) matmuls per location accumulate into PSUM.
  - One PSUM bank holds 8 locations; a single start=True on the first matmul
    clears the bank's has_written bits, later matmuls self-initialize their
    region (overwrite-where-unset, accumulate-where-set).
  - ScalarE applies ReLU PSUM->SBUF; out streams back as ot[i, a, j, r].
"""

import numpy as np

import concourse.bass as bass
import concourse.mybir as mybir
import concourse.tile as tile
from concourse import bacc
from concourse.bass_utils import run_bass_kernel_spmd

B = 64          # batch (= matmul N)
CIN = 64        # in channels
COUT = 64       # out channels (= matmul M)
H = 64
W = 64
KS = 3          # conv kernel size
NCORES = 8
RPC = H // NCORES        # output rows per core = 8
NPAIR = RPC // 2         # row pairs per core = 4
NPLANES = RPC + 2        # padded input planes per core = 10
NXP = NPLANES // 2       # paired x tiles = 5
WPAD = W + 2             # 66
NJQ = 4                  # j quarter-blocks
JQ = W // NJQ            # 16 columns per block
FP32 = mybir.dt.float32

_PROGRAM = None
LAST_RESULTS = None


def _build_program():
    """One Bass program, SPMD across 8 cores (inputs differ per core)."""
    nc = bacc.Bacc("TRN2", target_bir_lowering=False, debug=False,
                   num_devices=NCORES)
    # wt[t, jq, k(128), d(3), kind(3), j16, a] — see _pack_weights for k/kind.
    wt = nc.dram_tensor("wt", [NPAIR, NJQ, 128, KS, KS, JQ, COUT], FP32,
                        kind="ExternalInput")
    # xt[plane(10), b, r, v] — padded x planes for this core's rows.
    xt = nc.dram_tensor("xt", [NPLANES, CIN, B, WPAD], FP32,
                        kind="ExternalInput")
    # ot[il, a, j, r]
    ot = nc.dram_tensor("ot", [RPC, COUT, W, B], FP32, kind="ExternalOutput")

    with tile.TileContext(nc) as tc:
        with (
            tc.tile_pool(name="xpool", bufs=1) as xpool,
            tc.tile_pool(name="wpool", bufs=2) as wpool,
            tc.tile_pool(name="opool", bufs=2) as opool,
            tc.tile_pool(name="pspool", bufs=4,
                         space=bass.MemorySpace.PSUM) as pspool,
        ):
            # All x planes stay resident: 5 tiles [128=(plane parity, b), r, v].
            xp = []
            for s in range(NXP):
                t = xpool.tile([128, B, WPAD], FP32, tag=f"xp{s}")
                nc.sync.dma_start(
                    t[:], xt[2 * s:2 * s + 2].rearrange("p b r v -> (p b) r v"))
                xp.append(t)

            for tp in range(NPAIR):          # row pair: rows il = 2tp, 2tp+1
                for jq in range(NJQ):
                    wtile = wpool.tile([128, KS, KS, JQ, COUT], FP32, tag="w")
                    nc.sync.dma_start(wtile[:], wt[tp, jq])
                    o0 = opool.tile([COUT, JQ, B], FP32, tag="o0")
                    o1 = opool.tile([COUT, JQ, B], FP32, tag="o1")
                    for jb in range(2):      # 8-column PSUM banks
                        ps0 = pspool.tile([COUT, 8, B], FP32, tag="ps0")
                        ps1 = pspool.tile([COUT, 8, B], FP32, tag="ps1")
                        for d in range(KS):
                            for jj in range(8):
                                jl = jb * 8 + jj          # index into wtile j16
                                j = jq * JQ + jl          # global column
                                v = j + d                 # padded x column
                                first = (d == 0 and jj == 0)
                                last = (d == KS - 1 and jj == 7)
                                # row 2tp duals: c=0 (upper xp[tp]) + c=1 (lower)
                                nc.tensor.matmul(
                                    ps0[:, jj, :], wtile[:, d, 0, jl, :],
                                    xp[tp][:, :, v],
                                    start=first, stop=False)
                                # row 2tp+1 duals: c=1 (upper xp[tp+1]) + c=2
                                nc.tensor.matmul(
                                    ps1[:, jj, :], wtile[:, d, 1, jl, :],
                                    xp[tp + 1][:, :, v],
                                    start=first, stop=False)
                                # row 2tp single c=2: plane 2tp+2 = upper xp[tp+1]
                                nc.tensor.matmul(
                                    ps0[:, jj, :], wtile[0:64, d, 2, jl, :],
                                    xp[tp + 1][0:64, :, v],
                                    start=False, stop=last)
                                # row 2tp+1 single c=0: plane 2tp+1 = lower xp[tp]
                                nc.tensor.matmul(
                                    ps1[:, jj, :], wtile[64:128, d, 2, jl, :],
                                    xp[tp][64:128, :, v],
                                    start=False, stop=last)
                        nc.scalar.activation(
                            o0[:, jb * 8:jb * 8 + 8, :], ps0[:],
                            mybir.ActivationFunctionType.Relu)
                        nc.scalar.activation(
                            o1[:, jb * 8:jb * 8 + 8, :], ps1[:],
                            mybir.ActivationFunctionType.Relu)
                    nc.sync.dma_start(ot[2 * tp, :, jq * JQ:(jq + 1) * JQ, :], o0[:])
                    nc.sync.dma_start(ot[2 * tp + 1, :, jq * JQ:(jq + 1) * JQ, :], o1[:])
    nc.compile()
    return nc


def _pack_weights(weights):
    """weights (i, j, a, b, c, d) -> WH[T, jq, k, d, kind, j16, a] per row pair.

    kind 0 (row 2T duals):   k = c*64+b, c in {0,1}
    kind 1 (row 2T+1 duals): k = (c-1)*64+b, c in {1,2}
    kind 2 (singles):        k<64: (row 2T, c=2); k>=64: (row 2T+1, c=0)
    """
    wt6 = weights.transpose(0, 5, 4, 3, 1, 2)  # [i, d, c, b, j, a]
    even = wt6[0::2]                           # [32, d, c, b, j, a]
    odd = wt6[1::2]

    def stack_k(arr):  # [32, 3(d), 2(c), 64(b), 64(j), 64(a)] -> k-major
        a = arr.transpose(0, 2, 3, 1, 4, 5)    # [32, c, b, d, j, a]
        a = a.reshape(H // 2, 128, KS, NJQ, JQ, COUT)  # j -> (jq, j16)
        return a.transpose(0, 3, 1, 2, 4, 5)   # [32, jq, k, d, j16, a]

    d0 = stack_k(even[:, :, 0:2])
    d1 = stack_k(odd[:, :, 1:3])
    s = stack_k(np.concatenate([even[:, :, 2:3], odd[:, :, 0:1]], axis=2))
    # -> [32, jq, k, d, kind, j16, a]
    return np.ascontiguousarray(np.stack([d0, d1, s], axis=4))


def kernel(x, weights):
    global _PROGRAM, LAST_RESULTS
    x = np.ascontiguousarray(np.asarray(x, dtype=np.float32))
    weights = np.ascontiguousarray(np.asarray(weights, dtype=np.float32))
    assert x.shape == (B, CIN, H, W) and weights.shape == (H, W, COUT, CIN, KS, KS)

    xpad = np.pad(x, ((0, 0), (0, 0), (1, 1), (1, 1)))
    x_t = np.ascontiguousarray(xpad.transpose(2, 1, 0, 3))  # [u, b, r, v]
    wh = _pack_weights(weights)                             # [32, jq, k, d, e, j16, a]

    in_maps = []
    for cid in range(NCORES):
        in_maps.append({
            "wt": np.ascontiguousarray(wh[4 * cid:4 * cid + 4]),
            "xt": np.ascontiguousarray(x_t[RPC * cid:RPC * cid + NPLANES]),
        })

    if _PROGRAM is None:
        _PROGRAM = _build_program()
    res = run_bass_kernel_spmd(_PROGRAM, in_maps, list(range(NCORES)))
    LAST_RESULTS = res

    # ot[il, a, j, r] per core -> out[r, a, i, j]
    full = np.concatenate([res.results[c]["ot"] for c in range(NCORES)], axis=0)
    return np.ascontiguousarray(full.transpose(3, 1, 0, 2))


# revision 12
# speedup vs baseline: 1.8184x; 1.8184x over previous
"""Locally-connected conv (per-location weights) + ReLU on 8 Trainium2 cores.

Problem: x (B=64, Cin=64, H=64, W=64), weights (H, W, Cout=64, Cin=64, 3, 3)
  out[r,a,i,j] = relu( sum_{b,c,d} weights[i,j,a,b,c,d] * xpad[r,b,i+c,j+d] )

Sharding: data-parallel over H — core cid owns output rows i in [8*cid, 8*cid+8).
No collectives; pure SPMD with per-core input slices.

Device strategy (per core):
  - Host pre-packs weights into contraction-major tiles so every DMA has
    multi-KB contiguous partition lines (full HBM bandwidth).
  - x is padded/transposed on host to x_t[b, u, r, v] (u=h+1, v=w+1 padded
    planes); pairs of planes are stacked into 128-partition SBUF tiles so a
    single K=128 matmul contracts Cin x 2 vertical taps at once.
  - Per output row-pair and 16-column block: 3 dual-tap (K=128) + pairs of
    single-tap (K=64, opposite partition halves, run concurrently on the PE
    via row-group tiling) matmuls per location accumulate into PSUM.
  - One PSUM bank holds 8 locations; a single start=True on the first matmul
    clears the bank's has_written bits, later matmuls self-initialize their
    region (overwrite-where-unset, accumulate-where-set).
  - ScalarE applies ReLU PSUM->SBUF; out streams back as ot[i, a, j, r].
"""

import ml_dtypes
import numpy as np

import concourse.bass as bass
import concourse.mybir as mybir
import concourse.tile as tile
from concourse import bacc
from concourse.bass_utils import run_bass_kernel_spmd

B = 64          # batch (= matmul N)
CIN = 64        # in channels
COUT = 64       # out channels (= matmul M)
H = 64
W = 64
KS = 3          # conv kernel size
NCORES = 8
RPC = H // NCORES        # output rows per core = 8
NPAIR = RPC // 2         # row pairs per core = 4
NPLANES = RPC + 2        # padded input planes per core = 10
NXP = NPLANES // 2       # paired x tiles = 5
WPAD = W + 2             # 66
NJQ = 4                  # j quarter-blocks
JQ = W // NJQ            # 16 columns per block
FP32 = mybir.dt.float32
# bf16 inputs + fp32 PSUM accumulation: 4x PE throughput and half the HBM
# traffic vs fp32 (fp32 matmul lowers to 2 half-speed passes). Measured
# end-to-end max rel err ~2.5e-3.
CDT = mybir.dt.bfloat16
NP_CDT = ml_dtypes.bfloat16

_PROGRAM = None
LAST_RESULTS = None


def _build_program():
    """One Bass program, SPMD across 8 cores (inputs differ per core)."""
    nc = bacc.Bacc("TRN2", target_bir_lowering=False, debug=False,
                   num_devices=NCORES)
    # wt[t, jq, k(128), d(3), kind(3), j16, a] — see _pack_weights for k/kind.
    wt = nc.dram_tensor("wt", [NPAIR, NJQ, 128, KS, KS, JQ, COUT], CDT,
                        kind="ExternalInput")
    # xt[plane(10), b, r, v] — padded x planes for this core's rows.
    xt = nc.dram_tensor("xt", [NPLANES, CIN, B, WPAD], CDT,
                        kind="ExternalInput")
    # ot[il, a, j, r]
    ot = nc.dram_tensor("ot", [RPC, COUT, W, B], FP32, kind="ExternalOutput")

    with tile.TileContext(nc) as tc:
        with (
            tc.tile_pool(name="xpool", bufs=1) as xpool,
            tc.tile_pool(name="wpool", bufs=2) as wpool,
            tc.tile_pool(name="opool", bufs=2) as opool,
            tc.tile_pool(name="pspool", bufs=4,
                         space=bass.MemorySpace.PSUM) as pspool,
        ):
            # All x planes stay resident: 5 tiles [128=(plane parity, b), r, v].
            xp = []
            for s in range(NXP):
                t = xpool.tile([128, B, WPAD], CDT, tag=f"xp{s}")
                nc.sync.dma_start(
                    t[:], xt[2 * s:2 * s + 2].rearrange("p b r v -> (p b) r v"))
                xp.append(t)

            for tp in range(NPAIR):          # row pair: rows il = 2tp, 2tp+1
                for jq in range(NJQ):
                    wtile = wpool.tile([128, KS, KS, JQ, COUT], CDT, tag="w")
                    nc.sync.dma_start(wtile[:], wt[tp, jq])
                    o0 = opool.tile([COUT, JQ, B], FP32, tag="o0")
                    o1 = opool.tile([COUT, JQ, B], FP32, tag="o1")
                    for jb in range(2):      # 8-column PSUM banks
                        ps0 = pspool.tile([COUT, 8, B], FP32, tag="ps0")
                        ps1 = pspool.tile([COUT, 8, B], FP32, tag="ps1")
                        for d in range(KS):
                            for jj in range(8):
                                jl = jb * 8 + jj          # index into wtile j16
                                j = jq * JQ + jl          # global column
                                v = j + d                 # padded x column
                                first = (d == 0 and jj == 0)
                                last = (d == KS - 1 and jj == 7)
                                # row 2tp duals: c=0 (upper xp[tp]) + c=1 (lower)
                                nc.tensor.matmul(
                                    ps0[:, jj, :], wtile[:, d, 0, jl, :],
                                    xp[tp][:, :, v],
                                    start=first, stop=False)
                                # row 2tp+1 duals: c=1 (upper xp[tp+1]) + c=2
                                nc.tensor.matmul(
                                    ps1[:, jj, :], wtile[:, d, 1, jl, :],
                                    xp[tp + 1][:, :, v],
                                    start=first, stop=False)
                                # row 2tp single c=2: plane 2tp+2 = upper xp[tp+1]
                                nc.tensor.matmul(
                                    ps0[:, jj, :], wtile[0:64, d, 2, jl, :],
                                    xp[tp + 1][0:64, :, v],
                                    start=False, stop=last)
                                # row 2tp+1 single c=0: plane 2tp+1 = lower xp[tp]
                                nc.tensor.matmul(
                                    ps1[:, jj, :], wtile[64:128, d, 2, jl, :],
                                    xp[tp][64:128, :, v],
                                    start=False, stop=last)
                        nc.scalar.activation(
                            o0[:, jb * 8:jb * 8 + 8, :], ps0[:],
                            mybir.ActivationFunctionType.Relu)
                        nc.scalar.activation(
                            o1[:, jb * 8:jb * 8 + 8, :], ps1[:],
                            mybir.ActivationFunctionType.Relu)
                    nc.sync.dma_start(ot[2 * tp, :, jq * JQ:(jq + 1) * JQ, :], o0[:])
                    nc.sync.dma_start(ot[2 * tp + 1, :, jq * JQ:(jq + 1) * JQ, :], o1[:])
    nc.compile()
    return nc


def _pack_weights(weights):
    """weights (i, j, a, b, c, d) -> WH[T, jq, k, d, kind, j16, a] per row pair.

    kind 0 (row 2T duals):   k = c*64+b, c in {0,1}
    kind 1 (row 2T+1 duals): k = (c-1)*64+b, c in {1,2}
    kind 2 (singles):        k<64: (row 2T, c=2); k>=64: (row 2T+1, c=0)
    """
    wt6 = weights.transpose(0, 5, 4, 3, 1, 2)  # [i, d, c, b, j, a]
    even = wt6[0::2]                           # [32, d, c, b, j, a]
    odd = wt6[1::2]

    def stack_k(arr):  # [32, 3(d), 2(c), 64(b), 64(j), 64(a)] -> k-major
        a = arr.transpose(0, 2, 3, 1, 4, 5)    # [32, c, b, d, j, a]
        a = a.reshape(H // 2, 128, KS, NJQ, JQ, COUT)  # j -> (jq, j16)
        return a.transpose(0, 3, 1, 2, 4, 5)   # [32, jq, k, d, j16, a]

    d0 = stack_k(even[:, :, 0:2])
    d1 = stack_k(odd[:, :, 1:3])
    s = stack_k(np.concatenate([even[:, :, 2:3], odd[:, :, 0:1]], axis=2))
    # -> [32, jq, k, d, kind, j16, a]
    return np.ascontiguousarray(np.stack([d0, d1, s], axis=4))


def kernel(x, weights):
    global _PROGRAM, LAST_RESULTS
    x = np.ascontiguousarray(np.asarray(x, dtype=np.float32))
    weights = np.ascontiguousarray(np.asarray(weights, dtype=np.float32))
    assert x.shape == (B, CIN, H, W) and weights.shape == (H, W, COUT, CIN, KS, KS)

    xpad = np.pad(x, ((0, 0), (0, 0), (1, 1), (1, 1)))
    x_t = np.ascontiguousarray(xpad.transpose(2, 1, 0, 3))  # [u, b, r, v]
    wh = _pack_weights(weights)                             # [32, jq, k, d, e, j16, a]

    wh = wh.astype(NP_CDT)
    x_t = x_t.astype(NP_CDT)
    in_maps = []
    for cid in range(NCORES):
        in_maps.append({
            "wt": np.ascontiguousarray(wh[4 * cid:4 * cid + 4]),
            "xt": np.ascontiguousarray(x_t[RPC * cid:RPC * cid + NPLANES]),
        })

    if _PROGRAM is None:
        _PROGRAM = _build_program()
    res = run_bass_kernel_spmd(_PROGRAM, in_maps, list(range(NCORES)))
    LAST_RESULTS = res

    # ot[il, a, j, r] per core -> out[r, a, i, j]
    full = np.concatenate([res.results[c]["ot"] for c in range(NCORES)], axis=0)
    return np.ascontiguousarray(full.transpose(3, 1, 0, 2))


# revision 15
# speedup vs baseline: 2.3914x; 1.3152x over previous
"""Locally-connected conv (per-location weights) + ReLU on 8 Trainium2 cores.

Problem: x (B=64, Cin=64, H=64, W=64), weights (H, W, Cout=64, Cin=64, 3, 3)
  out[r,a,i,j] = relu( sum_{b,c,d} weights[i,j,a,b,c,d] * xpad[r,b,i+c,j+d] )

Sharding: data-parallel over H — core cid owns output rows i in [8*cid, 8*cid+8).
No collectives; pure SPMD with per-core input slices.

Device strategy (per core):
  - Host pre-packs weights into contraction-major tiles so every DMA has
    multi-KB contiguous partition lines (full HBM bandwidth).
  - x is padded/transposed on host to x_t[b, u, r, v] (u=h+1, v=w+1 padded
    planes); pairs of planes are stacked into 128-partition SBUF tiles so a
    single K=128 matmul contracts Cin x 2 vertical taps at once.
  - Per output row-pair and 16-column block: 3 dual-tap (K=128) + pairs of
    single-tap (K=64, opposite partition halves, run concurrently on the PE
    via row-group tiling) matmuls per location accumulate into PSUM.
  - One PSUM bank holds 8 locations; a single start=True on the first matmul
    clears the bank's has_written bits, later matmuls self-initialize their
    region (overwrite-where-unset, accumulate-where-set).
  - ScalarE applies ReLU PSUM->SBUF; out streams back as ot[i, a, j, r].
"""

import ml_dtypes
import numpy as np

import concourse.bass as bass
import concourse.mybir as mybir
import concourse.tile as tile
from concourse import bacc
from concourse.bass_utils import run_bass_kernel_spmd

B = 64          # batch (= matmul N)
CIN = 64        # in channels
COUT = 64       # out channels (= matmul M)
H = 64
W = 64
KS = 3          # conv kernel size
NCORES = 8
RPC = H // NCORES        # output rows per core = 8
NPAIR = RPC // 2         # row pairs per core = 4
NPLANES = RPC + 2        # padded input planes per core = 10
NXP = NPLANES // 2       # paired x tiles = 5
WPAD = W + 2             # 66
NJQ = 4                  # j quarter-blocks
JQ = W // NJQ            # 16 columns per block
FP32 = mybir.dt.float32
# bf16 inputs + fp32 PSUM accumulation: 4x PE throughput and half the HBM
# traffic vs fp32 (fp32 matmul lowers to 2 half-speed passes). Measured
# end-to-end max rel err ~2.5e-3.
CDT = mybir.dt.bfloat16
NP_CDT = ml_dtypes.bfloat16

_PROGRAM = None
LAST_RESULTS = None


def _build_program():
    """One Bass program, SPMD across 8 cores (inputs differ per core)."""
    nc = bacc.Bacc("TRN2", target_bir_lowering=False, debug=False,
                   num_devices=NCORES)
    # wt[t, jq, k(128), d(3), kind(3), j16, a] — see _pack_weights for k/kind.
    wt = nc.dram_tensor("wt", [NPAIR, NJQ, 128, KS, KS, JQ, COUT], CDT,
                        kind="ExternalInput")
    # xt[plane(10), b, v, r] — padded x planes for this core's rows.
    xt = nc.dram_tensor("xt", [NPLANES, CIN, WPAD, B], CDT,
                        kind="ExternalInput")
    # ot[il, a, j, r]
    ot = nc.dram_tensor("ot", [RPC, COUT, W, B], FP32, kind="ExternalOutput")

    with tile.TileContext(nc) as tc:
        with (
            tc.tile_pool(name="xpool", bufs=1) as xpool,
            tc.tile_pool(name="wpool", bufs=2) as wpool,
            tc.tile_pool(name="opool", bufs=2) as opool,
            tc.tile_pool(name="pspool", bufs=4,
                         space=bass.MemorySpace.PSUM) as pspool,
        ):
            # All x planes stay resident: 5 tiles [128=(plane parity, b), r, v].
            xp = []
            for s in range(NXP):
                # [128, v, r]: matmul rhs xp[:, v, :] streams contiguous columns
                t = xpool.tile([128, WPAD, B], CDT, tag=f"xp{s}")
                nc.sync.dma_start(
                    t[:], xt[2 * s:2 * s + 2].rearrange("p b v r -> (p b) v r"))
                xp.append(t)

            for tp in range(NPAIR):          # row pair: rows il = 2tp, 2tp+1
                for jq in range(NJQ):
                    wtile = wpool.tile([128, KS, KS, JQ, COUT], CDT, tag="w")
                    nc.sync.dma_start(wtile[:], wt[tp, jq])
                    o0 = opool.tile([COUT, JQ, B], FP32, tag="o0")
                    o1 = opool.tile([COUT, JQ, B], FP32, tag="o1")
                    for jb in range(2):      # 8-column PSUM banks
                        ps0 = pspool.tile([COUT, 8, B], FP32, tag="ps0")
                        ps1 = pspool.tile([COUT, 8, B], FP32, tag="ps1")
                        for d in range(KS):
                            for jj in range(8):
                                jl = jb * 8 + jj          # index into wtile j16
                                j = jq * JQ + jl          # global column
                                v = j + d                 # padded x column
                                first = (d == 0 and jj == 0)
                                last = (d == KS - 1 and jj == 7)
                                # row 2tp duals: c=0 (upper xp[tp]) + c=1 (lower)
                                nc.tensor.matmul(
                                    ps0[:, jj, :], wtile[:, d, 0, jl, :],
                                    xp[tp][:, v, :],
                                    start=first, stop=False)
                                # row 2tp+1 duals: c=1 (upper xp[tp+1]) + c=2
                                nc.tensor.matmul(
                                    ps1[:, jj, :], wtile[:, d, 1, jl, :],
                                    xp[tp + 1][:, v, :],
                                    start=first, stop=False)
                                # row 2tp single c=2: plane 2tp+2 = upper xp[tp+1]
                                nc.tensor.matmul(
                                    ps0[:, jj, :], wtile[0:64, d, 2, jl, :],
                                    xp[tp + 1][0:64, v, :],
                                    start=False, stop=last)
                                # row 2tp+1 single c=0: plane 2tp+1 = lower xp[tp]
                                nc.tensor.matmul(
                                    ps1[:, jj, :], wtile[64:128, d, 2, jl, :],
                                    xp[tp][64:128, v, :],
                                    start=False, stop=last)
                        nc.scalar.activation(
                            o0[:, jb * 8:jb * 8 + 8, :], ps0[:],
                            mybir.ActivationFunctionType.Relu)
                        nc.scalar.activation(
                            o1[:, jb * 8:jb * 8 + 8, :], ps1[:],
                            mybir.ActivationFunctionType.Relu)
                    nc.sync.dma_start(ot[2 * tp, :, jq * JQ:(jq + 1) * JQ, :], o0[:])
                    nc.sync.dma_start(ot[2 * tp + 1, :, jq * JQ:(jq + 1) * JQ, :], o1[:])
    nc.compile()
    return nc


def _pack_weights(weights):
    """weights (i, j, a, b, c, d) -> WH[T, jq, k, d, kind, j16, a] per row pair.

    kind 0 (row 2T duals):   k = c*64+b, c in {0,1}
    kind 1 (row 2T+1 duals): k = (c-1)*64+b, c in {1,2}
    kind 2 (singles):        k<64: (row 2T, c=2); k>=64: (row 2T+1, c=0)
    """
    wt6 = weights.transpose(0, 5, 4, 3, 1, 2)  # [i, d, c, b, j, a]
    even = wt6[0::2]                           # [32, d, c, b, j, a]
    odd = wt6[1::2]

    def stack_k(arr):  # [32, 3(d), 2(c), 64(b), 64(j), 64(a)] -> k-major
        a = arr.transpose(0, 2, 3, 1, 4, 5)    # [32, c, b, d, j, a]
        a = a.reshape(H // 2, 128, KS, NJQ, JQ, COUT)  # j -> (jq, j16)
        return a.transpose(0, 3, 1, 2, 4, 5)   # [32, jq, k, d, j16, a]

    d0 = stack_k(even[:, :, 0:2])
    d1 = stack_k(odd[:, :, 1:3])
    s = stack_k(np.concatenate([even[:, :, 2:3], odd[:, :, 0:1]], axis=2))
    # -> [32, jq, k, d, kind, j16, a]
    return np.ascontiguousarray(np.stack([d0, d1, s], axis=4))


def _prep_x(x):
    xpad = np.pad(x, ((0, 0), (0, 0), (1, 1), (1, 1)))
    return np.ascontiguousarray(xpad.transpose(2, 1, 3, 0))  # [u, b, v, r]


def kernel(x, weights):
    global _PROGRAM, LAST_RESULTS
    x = np.ascontiguousarray(np.asarray(x, dtype=np.float32))
    weights = np.ascontiguousarray(np.asarray(weights, dtype=np.float32))
    assert x.shape == (B, CIN, H, W) and weights.shape == (H, W, COUT, CIN, KS, KS)

    x_t = _prep_x(x)
    wh = _pack_weights(weights)                             # [32, jq, k, d, e, j16, a]

    wh = wh.astype(NP_CDT)
    x_t = x_t.astype(NP_CDT)
    in_maps = []
    for cid in range(NCORES):
        in_maps.append({
            "wt": np.ascontiguousarray(wh[4 * cid:4 * cid + 4]),
            "xt": np.ascontiguousarray(x_t[RPC * cid:RPC * cid + NPLANES]),
        })

    if _PROGRAM is None:
        _PROGRAM = _build_program()
    res = run_bass_kernel_spmd(_PROGRAM, in_maps, list(range(NCORES)))
    LAST_RESULTS = res

    # ot[il, a, j, r] per core -> out[r, a, i, j]
    full = np.concatenate([res.results[c]["ot"] for c in range(NCORES)], axis=0)
    return np.ascontiguousarray(full.transpose(3, 1, 0, 2))


# revision 18
# speedup vs baseline: 3.6589x; 1.5300x over previous
"""Locally-connected conv (per-location weights) + ReLU on 8 Trainium2 cores.

Problem: x (B=64, Cin=64, H=64, W=64), weights (H, W, Cout=64, Cin=64, 3, 3)
  out[r,a,i,j] = relu( sum_{b,c,d} weights[i,j,a,b,c,d] * xpad[r,b,i+c,j+d] )

Sharding: data-parallel over H — core cid owns output rows i in [8*cid, 8*cid+8).
No collectives; pure SPMD with per-core input slices.

Device strategy (per core):
  - Host pre-packs weights into contraction-major tiles so every DMA has
    multi-KB contiguous partition lines (full HBM bandwidth).
  - x is padded/transposed on host to x_t[b, u, r, v] (u=h+1, v=w+1 padded
    planes); pairs of planes are stacked into 128-partition SBUF tiles so a
    single K=128 matmul contracts Cin x 2 vertical taps at once.
  - Per output row-pair and 16-column block: 3 dual-tap (K=128) + pairs of
    single-tap (K=64, opposite partition halves, run concurrently on the PE
    via row-group tiling) matmuls per location accumulate into PSUM.
  - One PSUM bank holds 8 locations; a single start=True on the first matmul
    clears the bank's has_written bits, later matmuls self-initialize their
    region (overwrite-where-unset, accumulate-where-set).
  - ScalarE applies ReLU PSUM->SBUF; out streams back as ot[i, a, j, r].
"""

import ml_dtypes
import numpy as np

import concourse.bass as bass
import concourse.mybir as mybir
import concourse.tile as tile
from concourse import bacc
from concourse.bass_utils import run_bass_kernel_spmd

B = 64          # batch (= matmul N)
CIN = 64        # in channels
COUT = 64       # out channels (= matmul M)
H = 64
W = 64
KS = 3          # conv kernel size
NCORES = 8
RPC = H // NCORES        # output rows per core = 8
NPAIR = RPC // 2         # row pairs per core = 4
NPLANES = RPC + 2        # padded input planes per core = 10
NXP = NPLANES // 2       # paired x tiles = 5
WPAD = W + 2             # 66
NJQ = 4                  # j quarter-blocks
JQ = W // NJQ            # 16 columns per block
FP32 = mybir.dt.float32
# bf16 inputs + fp32 PSUM accumulation: 4x PE throughput and half the HBM
# traffic vs fp32 (fp32 matmul lowers to 2 half-speed passes). Measured
# end-to-end max rel err ~2.5e-3.
CDT = mybir.dt.bfloat16
NP_CDT = ml_dtypes.bfloat16

_PROGRAM = None
LAST_RESULTS = None


def _build_program():
    """One Bass program, SPMD across 8 cores (inputs differ per core)."""
    nc = bacc.Bacc("TRN2", target_bir_lowering=False, debug=False,
                   num_devices=NCORES)
    # wt[t, jq, k(128), d(3), kind(3), j16, a] — see _pack_weights for k/kind.
    wt = nc.dram_tensor("wt", [NPAIR, NJQ, 128, KS, KS, JQ, COUT], CDT,
                        kind="ExternalInput")
    # xt[plane(10), b, v, r] — padded x planes for this core's rows.
    xt = nc.dram_tensor("xt", [NPLANES, CIN, WPAD, B], CDT,
                        kind="ExternalInput")
    # ot[il, a, j, r]
    ot = nc.dram_tensor("ot", [RPC, COUT, W, B], FP32, kind="ExternalOutput")

    with tile.TileContext(nc) as tc:
        with (
            tc.tile_pool(name="xpool", bufs=1) as xpool,
            tc.tile_pool(name="wpool", bufs=2) as wpool,
            tc.tile_pool(name="opool", bufs=2) as opool,
            tc.tile_pool(name="pspool", bufs=2,
                         space=bass.MemorySpace.PSUM) as pspool,
        ):
            # All x planes stay resident: 5 tiles [128=(plane parity, b), r, v].
            xp = []
            for s in range(NXP):
                # [128, v, r]: matmul rhs xp[:, v, :] streams contiguous columns
                t = xpool.tile([128, WPAD, B], CDT, tag=f"xp{s}")
                nc.sync.dma_start(
                    t[:], xt[2 * s:2 * s + 2].rearrange("p b v r -> (p b) v r"))
                xp.append(t)

            for tp in range(NPAIR):          # row pair: rows il = 2tp, 2tp+1
                for jq in range(NJQ):
                    wtile = wpool.tile([128, KS, KS, JQ, COUT], CDT, tag="w")
                    nc.sync.dma_start(wtile[:], wt[tp, jq])
                    o0 = opool.tile([COUT, JQ, B], FP32, tag="o0")
                    o1 = opool.tile([COUT, JQ, B], FP32, tag="o1")
                    for jb in range(2):      # 8-column PSUM banks
                        # Each output row accumulates in TWO banks — one per
                        # PE row-group — so all K=64 matmuls on row-group 0
                        # run concurrently with the ones on row-group 64.
                        ps0a = pspool.tile([COUT, 8, B], FP32, tag="ps0a")
                        ps0b = pspool.tile([COUT, 8, B], FP32, tag="ps0b")
                        ps1a = pspool.tile([COUT, 8, B], FP32, tag="ps1a")
                        ps1b = pspool.tile([COUT, 8, B], FP32, tag="ps1b")
                        for d in range(KS):
                            for jj in range(8):
                                jl = jb * 8 + jj          # index into wtile j16
                                j = jq * JQ + jl          # global column
                                v = j + d                 # padded x column
                                first = (d == 0 and jj == 0)
                                last = (d == KS - 1 and jj == 7)
                                # row 2tp: c=0 (plane 2tp, rows 0-63 of xp[tp])
                                nc.tensor.matmul(
                                    ps0a[:, jj, :], wtile[0:64, d, 0, jl, :],
                                    xp[tp][0:64, v, :],
                                    start=first, stop=False)
                                # row 2tp: c=1 (plane 2tp+1, rows 64-127)
                                nc.tensor.matmul(
                                    ps0b[:, jj, :], wtile[64:128, d, 0, jl, :],
                                    xp[tp][64:128, v, :],
                                    start=first, stop=last)
                                # row 2tp+1: c=1 (plane 2tp+2, rows 0-63)
                                nc.tensor.matmul(
                                    ps1a[:, jj, :], wtile[0:64, d, 1, jl, :],
                                    xp[tp + 1][0:64, v, :],
                                    start=first, stop=last)
                                # row 2tp+1: c=2 (plane 2tp+3, rows 64-127)
                                nc.tensor.matmul(
                                    ps1b[:, jj, :], wtile[64:128, d, 1, jl, :],
                                    xp[tp + 1][64:128, v, :],
                                    start=first, stop=False)
                                # row 2tp single c=2: plane 2tp+2 = upper xp[tp+1]
                                nc.tensor.matmul(
                                    ps0a[:, jj, :], wtile[0:64, d, 2, jl, :],
                                    xp[tp + 1][0:64, v, :],
                                    start=False, stop=last)
                                # row 2tp+1 single c=0: plane 2tp+1 = lower xp[tp]
                                nc.tensor.matmul(
                                    ps1b[:, jj, :], wtile[64:128, d, 2, jl, :],
                                    xp[tp][64:128, v, :],
                                    start=False, stop=last)
                        # TensorTensor may read only ONE input from PSUM:
                        # ACT copies bank a, DVE adds bank b, ACT applies ReLU.
                        ob = jb * 8
                        s0 = o0[:, ob:ob + 8, :]
                        s1 = o1[:, ob:ob + 8, :]
                        nc.scalar.activation(
                            s0, ps0a[:], mybir.ActivationFunctionType.Copy)
                        nc.scalar.activation(
                            s1, ps1a[:], mybir.ActivationFunctionType.Copy)
                        nc.vector.tensor_add(s0, s0, ps0b[:])
                        nc.vector.tensor_add(s1, s1, ps1b[:])
                        nc.scalar.activation(
                            s0, s0, mybir.ActivationFunctionType.Relu)
                        nc.scalar.activation(
                            s1, s1, mybir.ActivationFunctionType.Relu)
                    nc.sync.dma_start(ot[2 * tp, :, jq * JQ:(jq + 1) * JQ, :], o0[:])
                    nc.sync.dma_start(ot[2 * tp + 1, :, jq * JQ:(jq + 1) * JQ, :], o1[:])
    nc.compile()
    return nc


def _pack_weights(weights):
    """weights (i, j, a, b, c, d) -> WH[T, jq, k, d, kind, j16, a] per row pair.

    kind 0 (row 2T duals):   k = c*64+b, c in {0,1}
    kind 1 (row 2T+1 duals): k = (c-1)*64+b, c in {1,2}
    kind 2 (singles):        k<64: (row 2T, c=2); k>=64: (row 2T+1, c=0)
    """
    wt6 = weights.transpose(0, 5, 4, 3, 1, 2)  # [i, d, c, b, j, a]
    even = wt6[0::2]                           # [32, d, c, b, j, a]
    odd = wt6[1::2]

    def stack_k(arr):  # [32, 3(d), 2(c), 64(b), 64(j), 64(a)] -> k-major
        a = arr.transpose(0, 2, 3, 1, 4, 5)    # [32, c, b, d, j, a]
        a = a.reshape(H // 2, 128, KS, NJQ, JQ, COUT)  # j -> (jq, j16)
        return a.transpose(0, 3, 1, 2, 4, 5)   # [32, jq, k, d, j16, a]

    d0 = stack_k(even[:, :, 0:2])
    d1 = stack_k(odd[:, :, 1:3])
    s = stack_k(np.concatenate([even[:, :, 2:3], odd[:, :, 0:1]], axis=2))
    # -> [32, jq, k, d, kind, j16, a]
    return np.ascontiguousarray(np.stack([d0, d1, s], axis=4))


def _prep_x(x):
    xpad = np.pad(x, ((0, 0), (0, 0), (1, 1), (1, 1)))
    return np.ascontiguousarray(xpad.transpose(2, 1, 3, 0))  # [u, b, v, r]


def kernel(x, weights):
    global _PROGRAM, LAST_RESULTS
    x = np.ascontiguousarray(np.asarray(x, dtype=np.float32))
    weights = np.ascontiguousarray(np.asarray(weights, dtype=np.float32))
    assert x.shape == (B, CIN, H, W) and weights.shape == (H, W, COUT, CIN, KS, KS)

    x_t = _prep_x(x)
    wh = _pack_weights(weights)                             # [32, jq, k, d, e, j16, a]

    wh = wh.astype(NP_CDT)
    x_t = x_t.astype(NP_CDT)
    in_maps = []
    for cid in range(NCORES):
        in_maps.append({
            "wt": np.ascontiguousarray(wh[4 * cid:4 * cid + 4]),
            "xt": np.ascontiguousarray(x_t[RPC * cid:RPC * cid + NPLANES]),
        })

    if _PROGRAM is None:
        _PROGRAM = _build_program()
    res = run_bass_kernel_spmd(_PROGRAM, in_maps, list(range(NCORES)))
    LAST_RESULTS = res

    # ot[il, a, j, r] per core -> out[r, a, i, j]
    full = np.concatenate([res.results[c]["ot"] for c in range(NCORES)], axis=0)
    return np.ascontiguousarray(full.transpose(3, 1, 0, 2))
